# revision 31
# baseline (speedup 1.0000x reference)
"""Trainium2 Bass kernel for nn_JointSelfAttentionLayer.

Math restructuring (both outputs are sequence-means):
  C[b]    = (1/(SC*sqrt(D))) * w_b @ x_d[b] @ W_vd,  w_b[t] = sum_s softmax(logits)[s,t]
  Dout[b] = (1/(SD*sqrt(D))) * (sum_s x_c[b,s,:]) @ W_vc   (softmax rows sum to 1)
with logits = x_c @ Wqk @ x_d^T, Wqk = W_qc @ W_kd^T. Never materializes
Q, K, V_c, V_d, or A@V.

All heavy matmuls run single-pass f16 (measured end-to-end rel err ~2e-3,
tolerance 2e-2): logit abs error ~0.03 vs logit std 32, and softmax colsum
averaging keeps the output error small. Data layout: x_c, x_d, W_qc, W_kd
are PE-transposed once (f32, psum->f16 copies); every GEMM then contracts
along natural partition dims:
  WqcT/WkdT -> Wqk (f16) -> GT = (x_c Wqk)^T via xcT -> L = GT^T-slices @ xdT
  per s-block: rowmax (DVE) -> exp+rowsum (ACT) -> cp += E/Z (DVE fused)
  w = colsum cp (gpsimd partition reduce), uT = <xdT, w> (DVE ttr),
  C = uT@Wvd, Dout = rT@Wvc (f16 matmuls, scaled 1/65536).

SBUF pools are a stack allocator: lifetimes are nested LIFO, peak
~189 KB/partition.
"""
import numpy as np
from contextlib import ExitStack

B, SC, SD, D = 8, 2048, 2048, 1024
P = 128
DB = D // P            # 8 d-blocks
CH = 512
NCH = SC // CH         # 4 chunks of 512 rows
SBK = SC // P          # 16 s-blocks
SCALE_OUT = 1.0 / (SC * 32.0)   # 1/(SC*sqrt(D)) == 1/(SD*sqrt(D))


def _split_excess_waits(nc, mybir, max_waits=1):
    n = 0
    ctr = [0]
    for fn in nc.m.functions:
        for bb in fn.blocks:
            out = []
            changed = False
            for inst in bb.instructions:
                si = inst.sync_info
                ws = list(si.on_wait) if (si and si.on_wait) else []
                if len(ws) > max_waits and inst.engine != mybir.EngineType.Unassigned:
                    keep = ws[:max_waits]
                    excess = ws[max_waits:]
                    for i in range(0, len(excess), max_waits):
                        chunk = excess[i:i + max_waits]
                        nop = mybir.InstNoOp(name=f"ws_{ctr[0]}", ins=[], outs=[])
                        ctr[0] += 1
                        nop.engine = inst.engine
                        nop.sync_info = mybir.SyncInfo(on_wait=chunk, on_update=[])
                        out.append(nop)
                    inst.sync_info = mybir.SyncInfo(
                        on_wait=keep, on_update=list(si.on_update or []))
                    changed = True
                    n += 1
                out.append(inst)
            if changed:
                bb.instructions = out
    return n


def _build(repeats=1):
    import concourse.bass as bass
    import concourse.tile as tile
    from concourse import mybir
    from concourse.masks import make_identity

    F32 = mybir.dt.float32
    F16 = mybir.dt.float16
    Act = mybir.ActivationFunctionType
    Alu = mybir.AluOpType
    AxX = mybir.AxisListType.X
    AxC = mybir.AxisListType.C

    nc = bass.Bass("TRN2", target_bir_lowering=False, debug=False, num_devices=8)
    xc = nc.dram_tensor("x_c", [SC, D], F32, kind="ExternalInput").ap()
    xd = nc.dram_tensor("x_d", [SD, D], F32, kind="ExternalInput").ap()
    wqc = nc.dram_tensor("W_qc", [D, D], F32, kind="ExternalInput").ap()
    wvc = nc.dram_tensor("W_vc", [D, D], F32, kind="ExternalInput").ap()
    wkd = nc.dram_tensor("W_kd", [D, D], F32, kind="ExternalInput").ap()
    wvd = nc.dram_tensor("W_vd", [D, D], F32, kind="ExternalInput").ap()
    out_d = nc.dram_tensor("out", [1, 2 * D], F32, kind="ExternalOutput").ap()

    with tile.TileContext(nc) as tc, ExitStack() as ctx:
        const = ctx.enter_context(tc.tile_pool(name="const", bufs=1))
        ident = const.tile([P, P], F32, name="ident")
        make_identity(nc, ident[:])
        cp = const.tile([P, SD], F32, name="cp")          # colsum partials
        # packed smalls: cols [0:8]=r (colsum x_c), [8:16]=uacc (w @ x_d)
        smalls = const.tile([P, 2 * DB], F32, name="smalls")
        s16 = const.tile([P, 2 * DB], F16, name="s16")    # f16 of the same
        out_sb = const.tile([1, 2 * D], F32, name="out_sb")
        ones32 = const.tile([P, 1], F32, name="ones32")
        nc.gpsimd.memset(ones32[:], 1.0)

        for _r in range(repeats):
            nc.gpsimd.memset(cp[:], 0.0)
            with tc.tile_pool(name=f"gtp_{_r}", bufs=1) as gt_p, \
                 tc.tile_pool(name=f"xdT_{_r}", bufs=1) as xdT_p:
                gt = [gt_p.tile([P, SC], F16, name=f"gt{j}_{_r}")
                      for j in range(DB)]
                xdT = [xdT_p.tile([P, SD], F16, name=f"xdT{j}_{_r}")
                       for j in range(DB)]

                with tc.tile_pool(name=f"xcT_{_r}", bufs=1) as xcT_p, \
                     tc.tile_pool(name=f"wqk16_{_r}", bufs=1) as wqk16_p, \
                     tc.tile_pool(name=f"raw_{_r}", bufs=2) as raw_p, \
                     tc.tile_pool(name=f"trps_{_r}", bufs=4, space="PSUM") as tr_ps, \
                     tc.tile_pool(name=f"mmps_{_r}", bufs=4, space="PSUM") as mm_ps:
                    xcT = [xcT_p.tile([P, SC], F16, name=f"xcT{j}_{_r}")
                           for j in range(DB)]
                    wqk16 = [wqk16_p.tile([P, D], F16, name=f"wqk{i}_{_r}")
                             for i in range(DB)]

                    # ---- W_qc/W_kd: load (sync ring), transpose f32 -> f16 ----
                    with tc.tile_pool(name=f"wT_{_r}", bufs=1) as wT_p:
                        wqcT = [wT_p.tile([P, D], F16, name=f"wqcT{k}_{_r}")
                                for k in range(DB)]
                        wkdT = [wT_p.tile([P, D], F16, name=f"wkdT{k}_{_r}")
                                for k in range(DB)]
                        for (dram, dstT, nm) in ((wqc, wqcT, "q"), (wkd, wkdT, "k")):
                            for c in range(2):
                                raw = raw_p.tile([P, 4, D], F32,
                                                 name=f"w{nm}raw{c}_{_r}", tag="raw")
                                nc.sync.dma_start(
                                    raw[:],
                                    dram[c * CH:(c + 1) * CH, :].rearrange(
                                        "(a p) e -> p a e", p=P))
                                for k in range(DB):
                                    tp = tr_ps.tile([P, CH], F32,
                                                    name=f"tw{nm}{c}{k}_{_r}",
                                                    tag="tp")
                                    for a in range(4):
                                        nc.tensor.transpose(
                                            tp[:, a * P:(a + 1) * P],
                                            raw[:, a, k * P:(k + 1) * P], ident[:])
                                    nc.scalar.activation(
                                        dstT[k][:, c * CH:(c + 1) * CH], tp[:],
                                        Act.Copy)

                        # ---- x_c: load (scalar ring), transpose -> xcT f16 ----
                        for c in range(NCH):
                            raw = raw_p.tile([P, 4, D], F32,
                                             name=f"xcraw{c}_{_r}", tag="raw")
                            nc.scalar.dma_start(
                                raw[:],
                                xc[c * CH:(c + 1) * CH, :].rearrange(
                                    "(a p) e -> p a e", p=P))
                            for j in range(DB):
                                tp = tr_ps.tile([P, CH], F32,
                                                name=f"txc{c}{j}_{_r}", tag="tp")
                                for a in range(4):
                                    nc.tensor.transpose(
                                        tp[:, a * P:(a + 1) * P],
                                        raw[:, a, j * P:(j + 1) * P], ident[:])
                                nc.scalar.activation(
                                    xcT[j][:, c * CH:(c + 1) * CH], tp[:],
                                    Act.Copy)

                        # ---- Wqk = Wqc @ Wkd^T (f16 single-pass) ----
                        # k-outer: lhsT loaded once per (i, k), reused 2x
                        for i in range(DB):
                            pss = [mm_ps.tile([P, CH], F32,
                                              name=f"pwqk{i}{cc}_{_r}", tag="pm")
                                   for cc in range(2)]
                            for k in range(DB):
                                for cc in range(2):
                                    nc.tensor.matmul(
                                        pss[cc][:], wqcT[k][:, i * P:(i + 1) * P],
                                        wkdT[k][:, cc * CH:(cc + 1) * CH],
                                        start=(k == 0), stop=(k == DB - 1))
                            for cc in range(2):
                                nc.scalar.activation(
                                    wqk16[i][:, cc * CH:(cc + 1) * CH],
                                    pss[cc][:], Act.Copy)

                    # ---- GT = (x_c @ Wqk)^T  [d2, s] ----
                    # i-outer: lhsT loaded once per (jp, i), reused 4x
                    for jp in range(DB):
                        pss = [mm_ps.tile([P, CH], F32,
                                          name=f"pgt{jp}{sc}_{_r}", tag="pm")
                               for sc in range(NCH)]
                        for i in range(DB):
                            for sc in range(NCH):
                                nc.tensor.matmul(
                                    pss[sc][:], wqk16[i][:, jp * P:(jp + 1) * P],
                                    xcT[i][:, sc * CH:(sc + 1) * CH],
                                    start=(i == 0), stop=(i == DB - 1))
                        for sc in range(NCH):
                            nc.vector.tensor_copy(
                                gt[jp][:, sc * CH:(sc + 1) * CH], pss[sc][:])

                    # r = colsum(x_c) from xcT (f16 -> f32 reduce)
                    for j in range(DB):
                        nc.vector.tensor_reduce(
                            smalls[:, j:j + 1], xcT[j][:], AxX, Alu.add)
                    nc.vector.tensor_copy(s16[:, 0:DB], smalls[:, 0:DB])

                    # ---- x_d: load, transpose -> xdT f16 ----
                    for c in range(NCH):
                        raw = raw_p.tile([P, 4, D], F32,
                                         name=f"xdraw{c}_{_r}", tag="raw")
                        nc.sync.dma_start(
                            raw[:],
                            xd[c * CH:(c + 1) * CH, :].rearrange(
                                "(a p) e -> p a e", p=P))
                        for j in range(DB):
                            tp = tr_ps.tile([P, CH], F32,
                                            name=f"txd{c}{j}_{_r}", tag="tp")
                            for a in range(4):
                                nc.tensor.transpose(
                                    tp[:, a * P:(a + 1) * P],
                                    raw[:, a, j * P:(j + 1) * P], ident[:])
                            nc.scalar.activation(
                                xdT[j][:, c * CH:(c + 1) * CH], tp[:], Act.Copy)

                # ---- W_vd/W_vc: casting DMA load (f32 DRAM -> f16 SBUF) ----
                with tc.tile_pool(name=f"wvp_{_r}", bufs=1) as wv_p:
                    wvdt = [wv_p.tile([P, 4, D], F16, name=f"wvd16{c}_{_r}")
                            for c in range(2)]
                    wvct = [wv_p.tile([P, 4, D], F16, name=f"wvc16{c}_{_r}")
                            for c in range(2)]
                    for (dram, dst, nm) in ((wvd, wvdt, "vd"), (wvc, wvct, "vc")):
                        for c in range(2):
                            nc.gpsimd.dma_start(
                                dst[c][:],
                                dram[c * CH:(c + 1) * CH, :].rearrange(
                                    "(a p) e -> p a e", p=P))
                    wvd16 = [wvdt[i // 4][:, i % 4, :] for i in range(DB)]
                    wvc16 = [wvct[i // 4][:, i % 4, :] for i in range(DB)]

                    # ---- epilogue broadcast tiles (late SBUF pool) ----
                    with tc.tile_pool(name=f"episb_{_r}", bufs=1) as epi_sb:
                        w16 = epi_sb.tile([1, SD], F16, name=f"w16_{_r}")
                        wB16 = epi_sb.tile([P, SD], F16, name=f"wB16_{_r}")
                        junk16 = [epi_sb.tile([P, SD], F16,
                                              name=f"junk16{k}_{_r}")
                                  for k in range(2)]
                        ones16 = epi_sb.tile([1, P], F16, name=f"ones16_{_r}")
                        nc.gpsimd.memset(ones16[:], 1.0)

                        # ---- logits + softmax colsum, per s-block ----
                        with tc.tile_pool(name=f"Lps_{_r}", bufs=2,
                                          space="PSUM") as L_ps, \
                             tc.tile_pool(name=f"E_{_r}", bufs=2) as E_p, \
                             tc.tile_pool(name=f"sm_{_r}", bufs=3) as sm_p:
                            for sb in range(SBK):
                                L = L_ps.tile([P, SD], F32, name=f"L{sb}_{_r}",
                                              tag="L")
                                # j-outer: lhsT loaded once per j, reused 4x
                                for j in range(DB):
                                    for tc_ in range(NCH):
                                        tsl = slice(tc_ * CH, (tc_ + 1) * CH)
                                        nc.tensor.matmul(
                                            L[:, tsl],
                                            gt[j][:, sb * P:(sb + 1) * P],
                                            xdT[j][:, tsl],
                                            start=(j == 0), stop=(j == DB - 1))
                                # smt cols: 0=-rowmax, 1=rowsum(exp), 2=1/rowsum
                                smt = sm_p.tile([P, 4], F32, name=f"smt{sb}_{_r}",
                                                tag="smt")
                                nc.vector.tensor_reduce(smt[:, 0:1], L[:], AxX,
                                                        Alu.max, negate=True)
                                E = E_p.tile([P, SD], F32, name=f"E{sb}_{_r}",
                                             tag="E")
                                nc.scalar.activation(E[:], L[:], Act.Exp,
                                                     bias=smt[:, 0:1], scale=1.0,
                                                     accum_out=smt[:, 1:2])
                                nc.vector.reciprocal(smt[:, 2:3], smt[:, 1:2])
                                nc.vector.scalar_tensor_tensor(
                                    cp[:], E[:], smt[:, 2:3], cp[:],
                                    Alu.mult, Alu.add)

                        # ---- epilogue ----
                        # w = colsum(cp) via ones-matmul (partition reduce)
                        with tc.tile_pool(name=f"wps_{_r}", bufs=1,
                                          space="PSUM") as wps_p:
                            wps = wps_p.tile([1, SD], F32, name=f"wps_{_r}")
                            for c in range(NCH):
                                nc.tensor.matmul(
                                    wps[:, c * CH:(c + 1) * CH], ones32[:],
                                    cp[:, c * CH:(c + 1) * CH],
                                    start=True, stop=True)
                            nc.scalar.activation(w16[:], wps[:], Act.Copy)
                        # broadcast w to all partitions via ones[1,P] matmul
                        with tc.tile_pool(name=f"wbps_{_r}", bufs=2,
                                          space="PSUM") as wb_ps:
                            for c in range(NCH):
                                ps = wb_ps.tile([P, CH], F32,
                                                name=f"wb{c}_{_r}", tag="wb")
                                nc.tensor.matmul(ps[:], ones16[:],
                                                 w16[:, c * CH:(c + 1) * CH],
                                                 start=True, stop=True)
                                nc.scalar.activation(
                                    wB16[:, c * CH:(c + 1) * CH], ps[:],
                                    Act.Copy)
                        # uT[d] = sum_t xdT[d,t] * w[t]  (DVE mult + reduce)
                        for i in range(DB):
                            jk = junk16[i % 2]
                            nc.vector.tensor_tensor(jk[:], xdT[i][:],
                                                    wB16[:], Alu.mult)
                            nc.vector.tensor_reduce(
                                smalls[:, DB + i:DB + i + 1], jk[:], AxX,
                                Alu.add)
                        nc.vector.tensor_copy(s16[:, DB:2 * DB],
                                              smalls[:, DB:2 * DB])

                        with tc.tile_pool(name=f"eps_{_r}", bufs=1,
                                          space="PSUM") as epi_ps:
                            pc = epi_ps.tile([1, D], F32, name=f"pc_{_r}")
                            pd = epi_ps.tile([1, D], F32, name=f"pd_{_r}")
                            for cc in range(2):
                                csl = slice(cc * CH, (cc + 1) * CH)
                                for i in range(DB):
                                    nc.tensor.matmul(
                                        pc[:, csl], s16[:, DB + i:DB + i + 1],
                                        wvd16[i][:, csl],
                                        start=(i == 0), stop=(i == DB - 1))
                                for i in range(DB):
                                    nc.tensor.matmul(
                                        pd[:, csl], s16[:, i:i + 1],
                                        wvc16[i][:, csl],
                                        start=(i == 0), stop=(i == DB - 1))
                            nc.scalar.activation(out_sb[:, 0:D], pc[:], Act.Copy,
                                                 scale=SCALE_OUT)
                            nc.scalar.activation(out_sb[:, D:2 * D], pd[:], Act.Copy,
                                                 scale=SCALE_OUT)
                        nc.sync.dma_start(out_d[:], out_sb[:])

    _split_excess_waits(nc, mybir)
    return nc


def kernel(x_c, x_d, W_qc, W_vc, W_kd, W_vd):
    from concourse.bass_utils import run_bass_kernel_spmd
    nc = _build()
    in_maps = []
    for b in range(B):
        in_maps.append({
            "x_c": np.ascontiguousarray(x_c[b]),
            "x_d": np.ascontiguousarray(x_d[b]),
            "W_qc": np.asarray(W_qc), "W_vc": np.asarray(W_vc),
            "W_kd": np.asarray(W_kd), "W_vd": np.asarray(W_vd),
        })
    res = run_bass_kernel_spmd(nc, in_maps, list(range(B))).results
    C = np.empty((B, D), dtype=np.float32)
    Dout = np.empty((B, D), dtype=np.float32)
    for b in range(B):
        o = res[b]["out"][0]
        C[b] = o[:D]
        Dout[b] = o[D:]
    return (C, Dout)


# revision 39
# speedup vs baseline: 1.0091x; 1.0091x over previous
"""Trainium2 Bass kernel for nn_JointSelfAttentionLayer.

Math restructuring (both outputs are sequence-means):
  C[b]    = (1/(SC*sqrt(D))) * w_b @ x_d[b] @ W_vd,  w_b[t] = sum_s softmax(logits)[s,t]
  Dout[b] = (1/(SD*sqrt(D))) * (sum_s x_c[b,s,:]) @ W_vc   (softmax rows sum to 1)
with logits = x_c @ Wqk @ x_d^T, Wqk = W_qc @ W_kd^T. Never materializes
Q, K, V_c, V_d, or A@V.

All heavy matmuls run single-pass f16 (measured end-to-end rel err ~2e-3,
tolerance 2e-2): logit abs error ~0.03 vs logit std 32, and softmax colsum
averaging keeps the output error small. Data layout: x_c, x_d, W_qc, W_kd
are PE-transposed once (f32, psum->f16 copies); every GEMM then contracts
along natural partition dims:
  WqcT/WkdT -> Wqk (f16) -> GT = (x_c Wqk)^T via xcT -> L = GT^T-slices @ xdT
  per s-block: rowmax (DVE) -> exp+rowsum (ACT) -> cp += E/Z (DVE fused)
  w = colsum cp (gpsimd partition reduce), uT = <xdT, w> (DVE ttr),
  C = uT@Wvd, Dout = rT@Wvc (f16 matmuls, scaled 1/65536).

SBUF pools are a stack allocator: lifetimes are nested LIFO, peak
~189 KB/partition.
"""
import numpy as np
from contextlib import ExitStack

B, SC, SD, D = 8, 2048, 2048, 1024
P = 128
DB = D // P            # 8 d-blocks
CH = 512
NCH = SC // CH         # 4 chunks of 512 rows
SBK = SC // P          # 16 s-blocks
SCALE_OUT = 1.0 / (SC * 32.0)   # 1/(SC*sqrt(D)) == 1/(SD*sqrt(D))


def _split_excess_waits(nc, mybir, max_waits=1):
    n = 0
    ctr = [0]
    for fn in nc.m.functions:
        for bb in fn.blocks:
            out = []
            changed = False
            for inst in bb.instructions:
                si = inst.sync_info
                ws = list(si.on_wait) if (si and si.on_wait) else []
                if len(ws) > max_waits and inst.engine != mybir.EngineType.Unassigned:
                    keep = ws[:max_waits]
                    excess = ws[max_waits:]
                    for i in range(0, len(excess), max_waits):
                        chunk = excess[i:i + max_waits]
                        nop = mybir.InstNoOp(name=f"ws_{ctr[0]}", ins=[], outs=[])
                        ctr[0] += 1
                        nop.engine = inst.engine
                        nop.sync_info = mybir.SyncInfo(on_wait=chunk, on_update=[])
                        out.append(nop)
                    inst.sync_info = mybir.SyncInfo(
                        on_wait=keep, on_update=list(si.on_update or []))
                    changed = True
                    n += 1
                out.append(inst)
            if changed:
                bb.instructions = out
    return n


def _build(repeats=1):
    import concourse.bass as bass
    import concourse.tile as tile
    from concourse import mybir
    from concourse.masks import make_identity

    F32 = mybir.dt.float32
    F16 = mybir.dt.float16
    Act = mybir.ActivationFunctionType
    Alu = mybir.AluOpType
    AxX = mybir.AxisListType.X
    AxC = mybir.AxisListType.C

    nc = bass.Bass("TRN2", target_bir_lowering=False, debug=False, num_devices=8)
    xc = nc.dram_tensor("x_c", [SC, D], F32, kind="ExternalInput").ap()
    xd = nc.dram_tensor("x_d", [SD, D], F32, kind="ExternalInput").ap()
    wqc = nc.dram_tensor("W_qc", [D, D], F32, kind="ExternalInput").ap()
    wvc = nc.dram_tensor("W_vc", [D, D], F32, kind="ExternalInput").ap()
    wkd = nc.dram_tensor("W_kd", [D, D], F32, kind="ExternalInput").ap()
    wvd = nc.dram_tensor("W_vd", [D, D], F32, kind="ExternalInput").ap()
    out_d = nc.dram_tensor("out", [1, 2 * D], F32, kind="ExternalOutput").ap()

    with tile.TileContext(nc) as tc, ExitStack() as ctx:
        const = ctx.enter_context(tc.tile_pool(name="const", bufs=1))
        ident = const.tile([P, P], F32, name="ident")
        make_identity(nc, ident[:])
        cp = const.tile([P, SD], F32, name="cp")          # colsum partials
        # packed smalls: cols [0:8]=r (colsum x_c), [8:16]=uacc (w @ x_d)
        smalls = const.tile([P, 2 * DB], F32, name="smalls")
        s16 = const.tile([P, 2 * DB], F16, name="s16")    # f16 of the same
        out_sb = const.tile([1, 2 * D], F32, name="out_sb")
        ones32 = const.tile([P, 1], F32, name="ones32")
        nc.gpsimd.memset(ones32[:], 1.0)

        for _r in range(repeats):
            nc.gpsimd.memset(cp[:], 0.0)
            with tc.tile_pool(name=f"gtp_{_r}", bufs=1) as gt_p, \
                 tc.tile_pool(name=f"xdT_{_r}", bufs=1) as xdT_p:
                gt = [gt_p.tile([P, SC], F16, name=f"gt{j}_{_r}")
                      for j in range(DB)]
                xdT = [xdT_p.tile([P, SD], F16, name=f"xdT{j}_{_r}")
                       for j in range(DB)]

                with tc.tile_pool(name=f"xcT_{_r}", bufs=1) as xcT_p, \
                     tc.tile_pool(name=f"wqk16_{_r}", bufs=1) as wqk16_p, \
                     tc.tile_pool(name=f"raw_{_r}", bufs=2) as raw_p, \
                     tc.tile_pool(name=f"trps_{_r}", bufs=4, space="PSUM") as tr_ps, \
                     tc.tile_pool(name=f"mmps_{_r}", bufs=4, space="PSUM") as mm_ps:
                    xcT = [xcT_p.tile([P, SC], F16, name=f"xcT{j}_{_r}")
                           for j in range(DB)]
                    wqk16 = [wqk16_p.tile([P, D], F16, name=f"wqk{i}_{_r}")
                             for i in range(DB)]

                    # ---- W_qc/W_kd: load (sync ring), transpose f32 -> f16 ----
                    with tc.tile_pool(name=f"wT_{_r}", bufs=1) as wT_p:
                        wqcT = [wT_p.tile([P, D], F16, name=f"wqcT{k}_{_r}")
                                for k in range(DB)]
                        wkdT = [wT_p.tile([P, D], F16, name=f"wkdT{k}_{_r}")
                                for k in range(DB)]
                        for (dram, dstT, nm) in ((wqc, wqcT, "q"), (wkd, wkdT, "k")):
                            for c in range(2):
                                raw = raw_p.tile([P, 4, D], F32,
                                                 name=f"w{nm}raw{c}_{_r}", tag="raw")
                                nc.sync.dma_start(
                                    raw[:],
                                    dram[c * CH:(c + 1) * CH, :].rearrange(
                                        "(a p) e -> p a e", p=P))
                                for k in range(DB):
                                    tp = tr_ps.tile([P, CH], F32,
                                                    name=f"tw{nm}{c}{k}_{_r}",
                                                    tag="tp")
                                    for a in range(4):
                                        nc.tensor.transpose(
                                            tp[:, a * P:(a + 1) * P],
                                            raw[:, a, k * P:(k + 1) * P], ident[:])
                                    nc.scalar.activation(
                                        dstT[k][:, c * CH:(c + 1) * CH], tp[:],
                                        Act.Copy)

                        # ---- x_c: load (scalar ring), transpose -> xcT f16 ----
                        for c in range(NCH):
                            raw = raw_p.tile([P, 4, D], F32,
                                             name=f"xcraw{c}_{_r}", tag="raw")
                            nc.scalar.dma_start(
                                raw[:],
                                xc[c * CH:(c + 1) * CH, :].rearrange(
                                    "(a p) e -> p a e", p=P))
                            for j in range(DB):
                                tp = tr_ps.tile([P, CH], F32,
                                                name=f"txc{c}{j}_{_r}", tag="tp")
                                for a in range(4):
                                    nc.tensor.transpose(
                                        tp[:, a * P:(a + 1) * P],
                                        raw[:, a, j * P:(j + 1) * P], ident[:])
                                nc.scalar.activation(
                                    xcT[j][:, c * CH:(c + 1) * CH], tp[:],
                                    Act.Copy)

                        # ---- Wqk = Wqc @ Wkd^T (f16 single-pass) ----
                        # k-outer: lhsT loaded once per (i, k), reused 2x
                        for i in range(DB):
                            pss = [mm_ps.tile([P, CH], F32,
                                              name=f"pwqk{i}{cc}_{_r}", tag="pm")
                                   for cc in range(2)]
                            for k in range(DB):
                                for cc in range(2):
                                    nc.tensor.matmul(
                                        pss[cc][:], wqcT[k][:, i * P:(i + 1) * P],
                                        wkdT[k][:, cc * CH:(cc + 1) * CH],
                                        start=(k == 0), stop=(k == DB - 1))
                            for cc in range(2):
                                nc.scalar.activation(
                                    wqk16[i][:, cc * CH:(cc + 1) * CH],
                                    pss[cc][:], Act.Copy)

                    # ---- GT = (x_c @ Wqk)^T  [d2, s] ----
                    # i-outer: lhsT loaded once per (jp, i), reused 4x
                    for jp in range(DB):
                        pss = [mm_ps.tile([P, CH], F32,
                                          name=f"pgt{jp}{sc}_{_r}", tag="pm")
                               for sc in range(NCH)]
                        for i in range(DB):
                            for sc in range(NCH):
                                nc.tensor.matmul(
                                    pss[sc][:], wqk16[i][:, jp * P:(jp + 1) * P],
                                    xcT[i][:, sc * CH:(sc + 1) * CH],
                                    start=(i == 0), stop=(i == DB - 1))
                        for sc in range(NCH):
                            nc.vector.tensor_copy(
                                gt[jp][:, sc * CH:(sc + 1) * CH], pss[sc][:])

                    # r = colsum(x_c) from xcT (f16 -> f32 reduce)
                    for j in range(DB):
                        nc.vector.tensor_reduce(
                            smalls[:, j:j + 1], xcT[j][:], AxX, Alu.add)
                    nc.vector.tensor_copy(s16[:, 0:DB], smalls[:, 0:DB])

                    # ---- x_d: load (scalar ring), transpose -> xdT f16 ----
                    for c in range(NCH):
                        raw = raw_p.tile([P, 4, D], F32,
                                         name=f"xdraw{c}_{_r}", tag="raw")
                        nc.scalar.dma_start(
                            raw[:],
                            xd[c * CH:(c + 1) * CH, :].rearrange(
                                "(a p) e -> p a e", p=P))
                        for j in range(DB):
                            tp = tr_ps.tile([P, CH], F32,
                                            name=f"txd{c}{j}_{_r}", tag="tp")
                            for a in range(4):
                                nc.tensor.transpose(
                                    tp[:, a * P:(a + 1) * P],
                                    raw[:, a, j * P:(j + 1) * P], ident[:])
                            nc.scalar.activation(
                                xdT[j][:, c * CH:(c + 1) * CH], tp[:], Act.Copy)

                # ---- W_vd/W_vc: casting DMA load (f32 DRAM -> f16 SBUF) ----
                with tc.tile_pool(name=f"wvp_{_r}", bufs=1) as wv_p:
                    wvdt = [wv_p.tile([P, 4, D], F16, name=f"wvd16{c}_{_r}")
                            for c in range(2)]
                    wvct = [wv_p.tile([P, 4, D], F16, name=f"wvc16{c}_{_r}")
                            for c in range(2)]
                    for (dram, dst, nm) in ((wvd, wvdt, "vd"), (wvc, wvct, "vc")):
                        for c in range(2):
                            nc.gpsimd.dma_start(
                                dst[c][:],
                                dram[c * CH:(c + 1) * CH, :].rearrange(
                                    "(a p) e -> p a e", p=P))
                    wvd16 = [wvdt[i // 4][:, i % 4, :] for i in range(DB)]
                    wvc16 = [wvct[i // 4][:, i % 4, :] for i in range(DB)]

                    # ---- epilogue broadcast tiles (late SBUF pool) ----
                    with tc.tile_pool(name=f"episb_{_r}", bufs=1) as epi_sb:
                        w16 = epi_sb.tile([1, SD], F16, name=f"w16_{_r}")
                        wB16 = epi_sb.tile([P, SD], F16, name=f"wB16_{_r}")
                        junk16 = [epi_sb.tile([P, SD], F16,
                                              name=f"junk16{k}_{_r}")
                                  for k in range(2)]
                        ones16 = epi_sb.tile([1, P], F16, name=f"ones16_{_r}")
                        nc.gpsimd.memset(ones16[:], 1.0)

                        # ---- logits + softmax colsum, per s-block ----
                        with tc.tile_pool(name=f"Lps_{_r}", bufs=2,
                                          space="PSUM") as L_ps, \
                             tc.tile_pool(name=f"E_{_r}", bufs=2) as E_p, \
                             tc.tile_pool(name=f"sm_{_r}", bufs=3) as sm_p:
                            for sb in range(SBK):
                                L = L_ps.tile([P, SD], F32, name=f"L{sb}_{_r}",
                                              tag="L")
                                # j-outer: lhsT loaded once per j, reused 4x
                                for j in range(DB):
                                    for tc_ in range(NCH):
                                        tsl = slice(tc_ * CH, (tc_ + 1) * CH)
                                        nc.tensor.matmul(
                                            L[:, tsl],
                                            gt[j][:, sb * P:(sb + 1) * P],
                                            xdT[j][:, tsl],
                                            start=(j == 0), stop=(j == DB - 1))
                                # smt cols: 0=-rowmax, 1=rowsum(exp), 2=1/rowsum
                                smt = sm_p.tile([P, 4], F32, name=f"smt{sb}_{_r}",
                                                tag="smt")
                                nc.vector.tensor_reduce(smt[:, 0:1], L[:], AxX,
                                                        Alu.max, negate=True)
                                E = E_p.tile([P, SD], F32, name=f"E{sb}_{_r}",
                                             tag="E")
                                nc.scalar.activation(E[:], L[:], Act.Exp,
                                                     bias=smt[:, 0:1], scale=1.0,
                                                     accum_out=smt[:, 1:2])
                                nc.vector.reciprocal(smt[:, 2:3], smt[:, 1:2])
                                nc.vector.scalar_tensor_tensor(
                                    cp[:], E[:], smt[:, 2:3], cp[:],
                                    Alu.mult, Alu.add)

                        # ---- epilogue ----
                        # w = colsum(cp) via ones-matmul (partition reduce)
                        with tc.tile_pool(name=f"wps_{_r}", bufs=1,
                                          space="PSUM") as wps_p:
                            wps = wps_p.tile([1, SD], F32, name=f"wps_{_r}")
                            for c in range(NCH):
                                nc.tensor.matmul(
                                    wps[:, c * CH:(c + 1) * CH], ones32[:],
                                    cp[:, c * CH:(c + 1) * CH],
                                    start=True, stop=True)
                            nc.scalar.activation(w16[:], wps[:], Act.Copy)
                        # broadcast w to all partitions via ones[1,P] matmul
                        with tc.tile_pool(name=f"wbps_{_r}", bufs=2,
                                          space="PSUM") as wb_ps:
                            for c in range(NCH):
                                ps = wb_ps.tile([P, CH], F32,
                                                name=f"wb{c}_{_r}", tag="wb")
                                nc.tensor.matmul(ps[:], ones16[:],
                                                 w16[:, c * CH:(c + 1) * CH],
                                                 start=True, stop=True)
                                nc.scalar.activation(
                                    wB16[:, c * CH:(c + 1) * CH], ps[:],
                                    Act.Copy)
                        # uT[d] = sum_t xdT[d,t] * w[t]  (DVE mult + reduce)
                        for i in range(DB):
                            jk = junk16[i % 2]
                            nc.vector.tensor_tensor(jk[:], xdT[i][:],
                                                    wB16[:], Alu.mult)
                            nc.vector.tensor_reduce(
                                smalls[:, DB + i:DB + i + 1], jk[:], AxX,
                                Alu.add)
                        nc.vector.tensor_copy(s16[:, DB:2 * DB],
                                              smalls[:, DB:2 * DB])

                        with tc.tile_pool(name=f"eps_{_r}", bufs=1,
                                          space="PSUM") as epi_ps:
                            pc = epi_ps.tile([1, D], F32, name=f"pc_{_r}")
                            pd = epi_ps.tile([1, D], F32, name=f"pd_{_r}")
                            for cc in range(2):
                                csl = slice(cc * CH, (cc + 1) * CH)
                                for i in range(DB):
                                    nc.tensor.matmul(
                                        pc[:, csl], s16[:, DB + i:DB + i + 1],
                                        wvd16[i][:, csl],
                                        start=(i == 0), stop=(i == DB - 1))
                                for i in range(DB):
                                    nc.tensor.matmul(
                                        pd[:, csl], s16[:, i:i + 1],
                                        wvc16[i][:, csl],
                                        start=(i == 0), stop=(i == DB - 1))
                            nc.scalar.activation(out_sb[:, 0:D], pc[:], Act.Copy,
                                                 scale=SCALE_OUT)
                            nc.scalar.activation(out_sb[:, D:2 * D], pd[:], Act.Copy,
                                                 scale=SCALE_OUT)
                        nc.sync.dma_start(out_d[:], out_sb[:])

    _split_excess_waits(nc, mybir)
    return nc


def kernel(x_c, x_d, W_qc, W_vc, W_kd, W_vd):
    from concourse.bass_utils import run_bass_kernel_spmd
    nc = _build()
    in_maps = []
    for b in range(B):
        in_maps.append({
            "x_c": np.ascontiguousarray(x_c[b]),
            "x_d": np.ascontiguousarray(x_d[b]),
            "W_qc": np.asarray(W_qc), "W_vc": np.asarray(W_vc),
            "W_kd": np.asarray(W_kd), "W_vd": np.asarray(W_vd),
        })
    res = run_bass_kernel_spmd(nc, in_maps, list(range(B))).results
    C = np.empty((B, D), dtype=np.float32)
    Dout = np.empty((B, D), dtype=np.float32)
    for b in range(B):
        o = res[b]["out"][0]
        C[b] = o[:D]
        Dout[b] = o[D:]
    return (C, Dout)


# revision 40
# speedup vs baseline: 1.0759x; 1.0662x over previous
"""Trainium2 Bass kernel for nn_JointSelfAttentionLayer.

Math restructuring (both outputs are sequence-means):
  C[b]    = (1/(SC*sqrt(D))) * w_b @ x_d[b] @ W_vd,  w_b[t] = sum_s softmax(logits)[s,t]
  Dout[b] = (1/(SD*sqrt(D))) * (sum_s x_c[b,s,:]) @ W_vc   (softmax rows sum to 1)
with logits = x_c @ Wqk @ x_d^T, Wqk = W_qc @ W_kd^T. Never materializes
Q, K, V_c, V_d, or A@V.

All heavy matmuls run single-pass f16 (measured end-to-end rel err ~2e-3,
tolerance 2e-2): logit abs error ~0.03 vs logit std 32, and softmax colsum
averaging keeps the output error small. Data layout: x_c, x_d, W_qc, W_kd
are PE-transposed once (f32, psum->f16 copies); every GEMM then contracts
along natural partition dims:
  WqcT/WkdT -> Wqk (f16) -> GT = (x_c Wqk)^T via xcT -> L = GT^T-slices @ xdT
  per s-block: rowmax (DVE) -> exp+rowsum (ACT) -> cp += E/Z (DVE fused)
  w = colsum cp (gpsimd partition reduce), uT = <xdT, w> (DVE ttr),
  C = uT@Wvd, Dout = rT@Wvc (f16 matmuls, scaled 1/65536).

SBUF pools are a stack allocator: lifetimes are nested LIFO, peak
~189 KB/partition.
"""
import numpy as np
from contextlib import ExitStack

B, SC, SD, D = 8, 2048, 2048, 1024
P = 128
DB = D // P            # 8 d-blocks
CH = 512
NCH = SC // CH         # 4 chunks of 512 rows
SBK = SC // P          # 16 s-blocks
SCALE_OUT = 1.0 / (SC * 32.0)   # 1/(SC*sqrt(D)) == 1/(SD*sqrt(D))


def _split_excess_waits(nc, mybir, max_waits=1):
    n = 0
    ctr = [0]
    for fn in nc.m.functions:
        for bb in fn.blocks:
            out = []
            changed = False
            for inst in bb.instructions:
                si = inst.sync_info
                ws = list(si.on_wait) if (si and si.on_wait) else []
                if len(ws) > max_waits and inst.engine != mybir.EngineType.Unassigned:
                    keep = ws[:max_waits]
                    excess = ws[max_waits:]
                    for i in range(0, len(excess), max_waits):
                        chunk = excess[i:i + max_waits]
                        nop = mybir.InstNoOp(name=f"ws_{ctr[0]}", ins=[], outs=[])
                        ctr[0] += 1
                        nop.engine = inst.engine
                        nop.sync_info = mybir.SyncInfo(on_wait=chunk, on_update=[])
                        out.append(nop)
                    inst.sync_info = mybir.SyncInfo(
                        on_wait=keep, on_update=list(si.on_update or []))
                    changed = True
                    n += 1
                out.append(inst)
            if changed:
                bb.instructions = out
    return n


def _build(repeats=1):
    import concourse.bass as bass
    import concourse.tile as tile
    from concourse import mybir
    from concourse.masks import make_identity

    F32 = mybir.dt.float32
    F16 = mybir.dt.float16
    Act = mybir.ActivationFunctionType
    Alu = mybir.AluOpType
    AxX = mybir.AxisListType.X
    AxC = mybir.AxisListType.C

    nc = bass.Bass("TRN2", target_bir_lowering=False, debug=False, num_devices=8)
    xc = nc.dram_tensor("x_c", [SC, D], F32, kind="ExternalInput").ap()
    xd = nc.dram_tensor("x_d", [SD, D], F32, kind="ExternalInput").ap()
    wqc = nc.dram_tensor("W_qc", [D, D], F32, kind="ExternalInput").ap()
    wvc = nc.dram_tensor("W_vc", [D, D], F32, kind="ExternalInput").ap()
    wkd = nc.dram_tensor("W_kd", [D, D], F32, kind="ExternalInput").ap()
    wvd = nc.dram_tensor("W_vd", [D, D], F32, kind="ExternalInput").ap()
    out_d = nc.dram_tensor("out", [1, 2 * D], F32, kind="ExternalOutput").ap()

    with tile.TileContext(nc) as tc, ExitStack() as ctx:
        const = ctx.enter_context(tc.tile_pool(name="const", bufs=1))
        ident = const.tile([P, P], F32, name="ident")
        make_identity(nc, ident[:])
        ident16 = const.tile([P, P], F16, name="ident16")
        make_identity(nc, ident16[:])
        cp = const.tile([P, SD], F32, name="cp")          # colsum partials
        # packed smalls: cols [0:8]=r (colsum x_c), [8:16]=uacc (w @ x_d)
        smalls = const.tile([P, 2 * DB], F32, name="smalls")
        s16 = const.tile([P, 2 * DB], F16, name="s16")    # f16 of the same
        out_sb = const.tile([1, 2 * D], F32, name="out_sb")
        ones32 = const.tile([P, 1], F32, name="ones32")
        nc.gpsimd.memset(ones32[:], 1.0)

        for _r in range(repeats):
            nc.gpsimd.memset(cp[:], 0.0)
            with tc.tile_pool(name=f"gtp_{_r}", bufs=1) as gt_p, \
                 tc.tile_pool(name=f"xdT_{_r}", bufs=1) as xdT_p:
                gt = [gt_p.tile([P, SC], F16, name=f"gt{j}_{_r}")
                      for j in range(DB)]
                xdT = [xdT_p.tile([P, SD], F16, name=f"xdT{j}_{_r}")
                       for j in range(DB)]

                with tc.tile_pool(name=f"xcT_{_r}", bufs=1) as xcT_p, \
                     tc.tile_pool(name=f"wqk16_{_r}", bufs=1) as wqk16_p, \
                     tc.tile_pool(name=f"raw_{_r}", bufs=2) as raw_p, \
                     tc.tile_pool(name=f"trps_{_r}", bufs=2, space="PSUM") as tr_ps, \
                     tc.tile_pool(name=f"tr16ps_{_r}", bufs=2, space="PSUM") as tr16_ps, \
                     tc.tile_pool(name=f"mmps_{_r}", bufs=4, space="PSUM") as mm_ps:
                    xcT = [xcT_p.tile([P, SC], F16, name=f"xcT{j}_{_r}")
                           for j in range(DB)]
                    wqk16 = [wqk16_p.tile([P, D], F16, name=f"wqk{i}_{_r}")
                             for i in range(DB)]

                    # ---- W_qc/W_kd: load (sync ring), transpose f32 -> f16 ----
                    with tc.tile_pool(name=f"wT_{_r}", bufs=1) as wT_p:
                        wqcT = [wT_p.tile([P, D], F16, name=f"wqcT{k}_{_r}")
                                for k in range(DB)]
                        wkdT = [wT_p.tile([P, D], F16, name=f"wkdT{k}_{_r}")
                                for k in range(DB)]
                        for (dram, dstT, nm) in ((wqc, wqcT, "q"), (wkd, wkdT, "k")):
                            for c in range(4):
                                raw = raw_p.tile([P, 2, D], F32,
                                                 name=f"w{nm}raw{c}_{_r}", tag="raw")
                                nc.sync.dma_start(
                                    raw[:],
                                    dram[c * 256:(c + 1) * 256, :].rearrange(
                                        "(a p) e -> p a e", p=P))
                                for k in range(DB):
                                    tp = tr_ps.tile([P, 256], F32,
                                                    name=f"tw{nm}{c}{k}_{_r}",
                                                    tag="tp")
                                    for a in range(2):
                                        nc.tensor.transpose(
                                            tp[:, a * P:(a + 1) * P],
                                            raw[:, a, k * P:(k + 1) * P], ident[:])
                                    nc.scalar.activation(
                                        dstT[k][:, c * 256:(c + 1) * 256], tp[:],
                                        Act.Copy)

                        # ---- x_c: load (scalar ring), transpose -> xcT f16 ----
                        for c in range(8):
                            raw = raw_p.tile([P, 2, D], F32,
                                             name=f"xcraw{c}_{_r}", tag="raw")
                            nc.scalar.dma_start(
                                raw[:],
                                xc[c * 256:(c + 1) * 256, :].rearrange(
                                    "(a p) e -> p a e", p=P))
                            for j in range(DB):
                                tp = tr_ps.tile([P, 256], F32,
                                                name=f"txc{c}{j}_{_r}", tag="tp")
                                for a in range(2):
                                    nc.tensor.transpose(
                                        tp[:, a * P:(a + 1) * P],
                                        raw[:, a, j * P:(j + 1) * P], ident[:])
                                nc.scalar.activation(
                                    xcT[j][:, c * 256:(c + 1) * 256], tp[:],
                                    Act.Copy)

                        # ---- x_d: casting DMA (gpsimd, f16) + f16 transposes ----
                        for c in range(NCH):
                            xdc = raw_p.tile([P, 4, D], F16,
                                             name=f"xdc{c}_{_r}", tag="xd16")
                            nc.gpsimd.dma_start(
                                xdc[:],
                                xd[c * CH:(c + 1) * CH, :].rearrange(
                                    "(a p) e -> p a e", p=P))
                            for j in range(DB):
                                tp16 = tr16_ps.tile([P, CH], F16,
                                                    name=f"txd{c}{j}_{_r}",
                                                    tag="tp16")
                                for a in range(4):
                                    nc.tensor.transpose(
                                        tp16[:, a * P:(a + 1) * P],
                                        xdc[:, a, j * P:(j + 1) * P], ident16[:])
                                nc.scalar.activation(
                                    xdT[j][:, c * CH:(c + 1) * CH], tp16[:],
                                    Act.Copy)

                        # ---- Wqk = Wqc @ Wkd^T (f16 single-pass) ----
                        # k-outer: lhsT loaded once per (i, k), reused 2x
                        for i in range(DB):
                            pss = [mm_ps.tile([P, CH], F32,
                                              name=f"pwqk{i}{cc}_{_r}", tag="pm")
                                   for cc in range(2)]
                            for k in range(DB):
                                for cc in range(2):
                                    nc.tensor.matmul(
                                        pss[cc][:], wqcT[k][:, i * P:(i + 1) * P],
                                        wkdT[k][:, cc * CH:(cc + 1) * CH],
                                        start=(k == 0), stop=(k == DB - 1))
                            for cc in range(2):
                                nc.scalar.activation(
                                    wqk16[i][:, cc * CH:(cc + 1) * CH],
                                    pss[cc][:], Act.Copy)

                    # ---- GT = (x_c @ Wqk)^T  [d2, s] ----
                    # i-outer: lhsT loaded once per (jp, i), reused 4x
                    for jp in range(DB):
                        pss = [mm_ps.tile([P, CH], F32,
                                          name=f"pgt{jp}{sc}_{_r}", tag="pm")
                               for sc in range(NCH)]
                        for i in range(DB):
                            for sc in range(NCH):
                                nc.tensor.matmul(
                                    pss[sc][:], wqk16[i][:, jp * P:(jp + 1) * P],
                                    xcT[i][:, sc * CH:(sc + 1) * CH],
                                    start=(i == 0), stop=(i == DB - 1))
                        for sc in range(NCH):
                            nc.vector.tensor_copy(
                                gt[jp][:, sc * CH:(sc + 1) * CH], pss[sc][:])

                    # r = colsum(x_c) from xcT (f16 -> f32 reduce)
                    for j in range(DB):
                        nc.vector.tensor_reduce(
                            smalls[:, j:j + 1], xcT[j][:], AxX, Alu.add)
                    nc.vector.tensor_copy(s16[:, 0:DB], smalls[:, 0:DB])

                # ---- W_vd/W_vc: casting DMA load (f32 DRAM -> f16 SBUF) ----
                with tc.tile_pool(name=f"wvp_{_r}", bufs=1) as wv_p:
                    wvdt = [wv_p.tile([P, 4, D], F16, name=f"wvd16{c}_{_r}")
                            for c in range(2)]
                    wvct = [wv_p.tile([P, 4, D], F16, name=f"wvc16{c}_{_r}")
                            for c in range(2)]
                    for (dram, dst, nm) in ((wvd, wvdt, "vd"), (wvc, wvct, "vc")):
                        for c in range(2):
                            nc.gpsimd.dma_start(
                                dst[c][:],
                                dram[c * CH:(c + 1) * CH, :].rearrange(
                                    "(a p) e -> p a e", p=P))
                    wvd16 = [wvdt[i // 4][:, i % 4, :] for i in range(DB)]
                    wvc16 = [wvct[i // 4][:, i % 4, :] for i in range(DB)]

                    # ---- epilogue broadcast tiles (late SBUF pool) ----
                    with tc.tile_pool(name=f"episb_{_r}", bufs=1) as epi_sb:
                        w16 = epi_sb.tile([1, SD], F16, name=f"w16_{_r}")
                        wB16 = epi_sb.tile([P, SD], F16, name=f"wB16_{_r}")
                        junk16 = [epi_sb.tile([P, SD], F16,
                                              name=f"junk16{k}_{_r}")
                                  for k in range(2)]
                        ones16 = epi_sb.tile([1, P], F16, name=f"ones16_{_r}")
                        nc.gpsimd.memset(ones16[:], 1.0)

                        # ---- logits + softmax colsum, per s-block ----
                        with tc.tile_pool(name=f"Lps_{_r}", bufs=2,
                                          space="PSUM") as L_ps, \
                             tc.tile_pool(name=f"E_{_r}", bufs=3) as E_p, \
                             tc.tile_pool(name=f"sm_{_r}", bufs=3) as sm_p:
                            for sb in range(SBK):
                                L = L_ps.tile([P, SD], F32, name=f"L{sb}_{_r}",
                                              tag="L")
                                # j-outer: lhsT loaded once per j, reused 4x
                                for j in range(DB):
                                    for tc_ in range(NCH):
                                        tsl = slice(tc_ * CH, (tc_ + 1) * CH)
                                        nc.tensor.matmul(
                                            L[:, tsl],
                                            gt[j][:, sb * P:(sb + 1) * P],
                                            xdT[j][:, tsl],
                                            start=(j == 0), stop=(j == DB - 1))
                                # smt cols: 0=-rowmax, 1=rowsum(exp), 2=1/rowsum
                                smt = sm_p.tile([P, 4], F32, name=f"smt{sb}_{_r}",
                                                tag="smt")
                                nc.vector.tensor_reduce(smt[:, 0:1], L[:], AxX,
                                                        Alu.max, negate=True)
                                E = E_p.tile([P, SD], F32, name=f"E{sb}_{_r}",
                                             tag="E")
                                nc.scalar.activation(E[:], L[:], Act.Exp,
                                                     bias=smt[:, 0:1], scale=1.0,
                                                     accum_out=smt[:, 1:2])
                                nc.vector.reciprocal(smt[:, 2:3], smt[:, 1:2])
                                nc.vector.scalar_tensor_tensor(
                                    cp[:], E[:], smt[:, 2:3], cp[:],
                                    Alu.mult, Alu.add)

                        # ---- epilogue ----
                        # w = colsum(cp) via ones-matmul (partition reduce)
                        with tc.tile_pool(name=f"wps_{_r}", bufs=1,
                                          space="PSUM") as wps_p:
                            wps = wps_p.tile([1, SD], F32, name=f"wps_{_r}")
                            for c in range(NCH):
                                nc.tensor.matmul(
                                    wps[:, c * CH:(c + 1) * CH], ones32[:],
                                    cp[:, c * CH:(c + 1) * CH],
                                    start=True, stop=True)
                            nc.scalar.activation(w16[:], wps[:], Act.Copy)
                        # broadcast w to all partitions via ones[1,P] matmul
                        with tc.tile_pool(name=f"wbps_{_r}", bufs=2,
                                          space="PSUM") as wb_ps:
                            for c in range(NCH):
                                ps = wb_ps.tile([P, CH], F32,
                                                name=f"wb{c}_{_r}", tag="wb")
                                nc.tensor.matmul(ps[:], ones16[:],
                                                 w16[:, c * CH:(c + 1) * CH],
                                                 start=True, stop=True)
                                nc.scalar.activation(
                                    wB16[:, c * CH:(c + 1) * CH], ps[:],
                                    Act.Copy)
                        # uT[d] = sum_t xdT[d,t] * w[t]  (DVE mult + reduce)
                        for i in range(DB):
                            jk = junk16[i % 2]
                            nc.vector.tensor_tensor(jk[:], xdT[i][:],
                                                    wB16[:], Alu.mult)
                            nc.vector.tensor_reduce(
                                smalls[:, DB + i:DB + i + 1], jk[:], AxX,
                                Alu.add)
                        nc.vector.tensor_copy(s16[:, DB:2 * DB],
                                              smalls[:, DB:2 * DB])

                        with tc.tile_pool(name=f"eps_{_r}", bufs=1,
                                          space="PSUM") as epi_ps:
                            pc = epi_ps.tile([1, D], F32, name=f"pc_{_r}")
                            pd = epi_ps.tile([1, D], F32, name=f"pd_{_r}")
                            for cc in range(2):
                                csl = slice(cc * CH, (cc + 1) * CH)
                                for i in range(DB):
                                    nc.tensor.matmul(
                                        pc[:, csl], s16[:, DB + i:DB + i + 1],
                                        wvd16[i][:, csl],
                                        start=(i == 0), stop=(i == DB - 1))
                                for i in range(DB):
                                    nc.tensor.matmul(
                                        pd[:, csl], s16[:, i:i + 1],
                                        wvc16[i][:, csl],
                                        start=(i == 0), stop=(i == DB - 1))
                            nc.scalar.activation(out_sb[:, 0:D], pc[:], Act.Copy,
                                                 scale=SCALE_OUT)
                            nc.scalar.activation(out_sb[:, D:2 * D], pd[:], Act.Copy,
                                                 scale=SCALE_OUT)
                        nc.sync.dma_start(out_d[:], out_sb[:])

    _split_excess_waits(nc, mybir)
    return nc


def kernel(x_c, x_d, W_qc, W_vc, W_kd, W_vd):
    from concourse.bass_utils import run_bass_kernel_spmd
    nc = _build()
    in_maps = []
    for b in range(B):
        in_maps.append({
            "x_c": np.ascontiguousarray(x_c[b]),
            "x_d": np.ascontiguousarray(x_d[b]),
            "W_qc": np.asarray(W_qc), "W_vc": np.asarray(W_vc),
            "W_kd": np.asarray(W_kd), "W_vd": np.asarray(W_vd),
        })
    res = run_bass_kernel_spmd(nc, in_maps, list(range(B))).results
    C = np.empty((B, D), dtype=np.float32)
    Dout = np.empty((B, D), dtype=np.float32)
    for b in range(B):
        o = res[b]["out"][0]
        C[b] = o[:D]
        Dout[b] = o[D:]
    return (C, Dout)
